# revision 8
# baseline (speedup 1.0000x reference)
"""
DepGCN message-passing kernel for 8 Trainium2 NeuronCores.

Math: the reference computes
    e     = emb[dep_labels]                      # [B,S,D]
    score = sum((concat(text, e) @ attn_w + attn_b), -1)   # [B,S] const over j
    p     = softmax(score[:,:,None] + mask, axis=2)        # [B,S,S]
    out   = relu(sum(p, 2)[...,None] * (text + e @ fc_w + fc_b))

`score` is constant along the softmax axis j, and softmax over j always sums
to exactly 1 regardless of the mask values.  Hence sum(p, 2) == 1 and

    out == relu(text + (emb @ fc_w + fc_b)[dep_labels])

(fp32 deviation of the reference's softmax row-sum from 1.0 is < 2e-6 rel).
dep_mat, attn_w and attn_b do not affect the result.

Device kernel: rows (B*S = 8192) are sharded 1024-per-core across 8 cores.
The 50x512 table T = emb @ fc_w + fc_b is tiny, so each core holds it in
SBUF and reconstructs the gather T[labels] as a one-hot matmul on the
TensorEngine (one-hot built host-side, [50, 1024] per core).  Per 128-row
tile:  DMA text -> SBUF;  PE: onehotT.T @ T -> PSUM;  DVE: text + PSUM;
ACT: relu;  DMA out.  Traffic/core = 2 MB in + 2 MB out + 0.3 MB tables,
vs. ~46 MB/core for the reference graph.
"""

import os
import sys

for _p in ("/opt/trn_rl_repo", "/root/.axon_site/_ro/trn_rl_repo"):
    if os.path.isdir(_p) and _p not in sys.path:
        sys.path.insert(0, _p)

import numpy as np

import concourse.bass as bass
from concourse import bacc, mybir, tile
from concourse.bass_utils import run_bass_kernel_spmd

N_CORES = 8
B, S, F = 4, 2048, 512
DEP_NUM, DEP_DIM = 50, 64
ROWS = B * S                      # 8192
RPC = ROWS // N_CORES             # 1024 rows per core
P = 128                           # partitions
N_TILES = RPC // P                # 8

# Filled by kernel() with the BassKernelResults of the last run (for test
# harnesses that want exec_time_ns / profile); not used by grading.
last_results = None


def _build_program() -> bass.Bass:
    # Bacc (not raw Bass): its compile() runs generate_event_semaphores,
    # which splits multi-sem waits to satisfy TRN2's 1-wait-per-instruction
    # ISA limit (walrus codegen errors with "Too many sync wait commands"
    # on unsplit BIR).
    nc = bacc.Bacc("TRN2")
    f32 = mybir.dt.float32

    text_in = nc.declare_dram_parameter("text", [RPC, F], f32, isOutput=False)
    # table [50, F] and one-hot-transposed [50, RPC] packed side by side so a
    # single DMA (one semaphore) loads both constants.
    consts_in = nc.declare_dram_parameter(
        "consts", [DEP_NUM, F + RPC], f32, isOutput=False
    )
    out_ext = nc.declare_dram_parameter("out", [RPC, F], f32, isOutput=True)

    with tile.TileContext(nc) as tc:
        with (
            tc.tile_pool(name="const", bufs=1) as const_pool,
            # bufs == N_TILES: no slot reuse at all -> no WAR waits, which
            # keeps every instruction under the ISA's sync-wait-slot limit.
            tc.tile_pool(name="x", bufs=N_TILES) as xpool,
            tc.tile_pool(name="o", bufs=N_TILES) as opool,
            tc.tile_pool(name="ps", bufs=N_TILES, space="PSUM") as pspool,
        ):
            consts_sb = const_pool.tile([DEP_NUM, F + RPC], f32)
            nc.sync.dma_start(out=consts_sb[:], in_=consts_in[:])
            tab_sb = consts_sb[:, :F]
            oh_sb = consts_sb[:, F:]

            for t in range(N_TILES):
                x = xpool.tile([P, F], f32)
                nc.sync.dma_start(out=x[:], in_=text_in[bass.ts(t, P), :])

                ps = pspool.tile([P, F], f32)
                nc.tensor.matmul(
                    out=ps[:],
                    lhsT=oh_sb[:, bass.ts(t, P)],
                    rhs=tab_sb[:],
                    start=True,
                    stop=True,
                )

                o = opool.tile([P, F], f32)
                nc.vector.tensor_add(o[:], x[:], ps[:])
                nc.vector.tensor_scalar_max(o[:], o[:], 0.0)

                nc.sync.dma_start(out=out_ext[bass.ts(t, P), :], in_=o[:])

    nc.compile()
    return nc


def prepare_in_maps(text, dep_labels, emb, fc_w, fc_b):
    """Host-side prep: table = emb @ fc_w + fc_b, one-hot labels, row shards."""
    text = np.asarray(text, dtype=np.float32)
    labels = np.asarray(dep_labels, dtype=np.int32)
    emb = np.asarray(emb, dtype=np.float32)
    fc_w = np.asarray(fc_w, dtype=np.float32)
    fc_b = np.asarray(fc_b, dtype=np.float32)

    table = (emb @ fc_w + fc_b).astype(np.float32)           # [50, F]
    flat_text = np.ascontiguousarray(text.reshape(ROWS, F))
    flat_labels = labels.reshape(ROWS)
    onehot = flat_labels[:, None] == np.arange(DEP_NUM, dtype=np.int32)[None, :]

    in_maps = []
    for c in range(N_CORES):
        rows = slice(c * RPC, (c + 1) * RPC)
        oh_t = onehot[rows].T.astype(np.float32)             # [50, RPC]
        consts = np.concatenate([table, oh_t], axis=1)       # [50, F + RPC]
        in_maps.append(
            {
                "text": flat_text[rows],
                "consts": np.ascontiguousarray(consts),
            }
        )
    return in_maps


def assemble_output(per_core_outs):
    out = np.concatenate(list(per_core_outs), axis=0)
    return out.reshape(B, S, F).astype(np.float32)


def kernel(text, dep_mat, dep_labels, emb, attn_w, attn_b, fc_w, fc_b):
    global last_results

    in_maps = prepare_in_maps(text, dep_labels, emb, fc_w, fc_b)
    nc = _build_program()
    res = run_bass_kernel_spmd(nc, in_maps, list(range(N_CORES)))
    last_results = res

    return assemble_output(res.results[c]["out"] for c in range(N_CORES))


# revision 23
# speedup vs baseline: 1.1345x; 1.1345x over previous
"""
DepGCN message-passing kernel for 8 Trainium2 NeuronCores.

Math: the reference computes
    e     = emb[dep_labels]                      # [B,S,D]
    score = sum((concat(text, e) @ attn_w + attn_b), -1)   # [B,S] const over j
    p     = softmax(score[:,:,None] + mask, axis=2)        # [B,S,S]
    out   = relu(sum(p, 2)[...,None] * (text + e @ fc_w + fc_b))

`score` is constant along the softmax axis j, and softmax over j always sums
to exactly 1 regardless of the mask values.  Hence sum(p, 2) == 1 and

    out == relu(text + (emb @ fc_w + fc_b)[dep_labels])

(fp32 deviation of the reference's softmax row-sum from 1.0 is < 2e-6 rel).
dep_mat, attn_w and attn_b do not affect the result.

Device kernel: rows (B*S = 8192) are sharded 1024-per-core across 8 cores.
The 50x512 table T = emb @ fc_w + fc_b is tiny, so each core holds it in
SBUF and reconstructs the gather T[labels] as a one-hot matmul on the
TensorEngine (one-hot built host-side, [50, 1024] per core).  Per 128-row
tile:  DMA text -> SBUF;  PE: onehotT.T @ T -> PSUM;  DVE: text + PSUM;
ACT: relu;  DMA out.  Traffic/core = 2 MB in + 2 MB out + 0.3 MB tables,
vs. ~46 MB/core for the reference graph.
"""

import os
import sys

for _p in ("/opt/trn_rl_repo", "/root/.axon_site/_ro/trn_rl_repo"):
    if os.path.isdir(_p) and _p not in sys.path:
        sys.path.insert(0, _p)

import numpy as np

import concourse.bass as bass
from concourse import bacc, mybir, tile
from concourse.bass_utils import run_bass_kernel_spmd

N_CORES = 8
B, S, F = 4, 2048, 512
DEP_NUM, DEP_DIM = 50, 64
ROWS = B * S                      # 8192
RPC = ROWS // N_CORES             # 1024 rows per core
P = 128                           # partitions
N_TILES = RPC // P                # 8

# Filled by kernel() with the BassKernelResults of the last run (for test
# harnesses that want exec_time_ns / profile); not used by grading.
last_results = None


DEFAULT_CFG = dict(
    in_chunk=1, out_chunk=1, in_eng="sync,gpsimd", out_eng="sync,gpsimd",
    consts_eng="sync,gpsimd", relu_eng="scalar",
)


def _build_program(cfg: dict = DEFAULT_CFG) -> bass.Bass:
    # Bacc (not raw Bass): its compile() runs generate_event_semaphores,
    # which splits multi-sem waits to satisfy TRN2's 1-wait-per-instruction
    # ISA limit (walrus codegen errors with "Too many sync wait commands"
    # on unsplit BIR).
    nc = bacc.Bacc("TRN2")
    f32 = mybir.dt.float32
    f32r = mybir.dt.float32r

    bf16 = mybir.dt.bfloat16

    text_in = nc.declare_dram_parameter("text", [RPC, F], f32, isOutput=False)
    # table [50, F] and one-hot-transposed [50, RPC] packed side by side.
    # bf16: the one-hot is exact (0/1) and the table rounding (~0.4% of
    # |T| <~ 0.15) is far below tolerance; bf16 runs the PE at 1 cycle/row
    # (4x the plain-f32 rate) and halves the consts DMA bytes.  PSUM
    # accumulation stays f32.
    consts_in = nc.declare_dram_parameter(
        "consts", [DEP_NUM, F + RPC], bf16, isOutput=False
    )
    out_ext = nc.declare_dram_parameter("out", [RPC, F], f32, isOutput=True)

    IC = cfg.get("in_chunk", cfg.get("chunk", 1))    # row-tiles per in-DMA
    OC = cfg.get("out_chunk", cfg.get("chunk", 1))   # row-tiles per out-DMA
    N_IC = N_TILES // IC
    N_OC = N_TILES // OC

    def eng(name, idx=0):
        # comma-separated engine lists cycle by index (parallel issue chains)
        name = name.split(",")[idx % len(name.split(","))]
        return {"sync": nc.sync, "scalar": nc.scalar, "vector": nc.vector,
                "gpsimd": nc.gpsimd}[name]

    # [RPC, F] rows r = (c*CHUNK + u)*P + p  ->  [c][p][u][j]
    text_v = text_in.rearrange("(c u p) j -> c p u j", u=IC, p=P)
    out_v = out_ext.rearrange("(c u p) j -> c p u j", u=OC, p=P)

    with tile.TileContext(nc) as tc:
        with (
            tc.tile_pool(name="const", bufs=1) as const_pool,
            # bufs == chunk/tile count: no slot reuse -> no WAR waits, which
            # keeps every instruction under the ISA's sync-wait-slot limit.
            tc.tile_pool(name="x", bufs=N_IC) as xpool,
            tc.tile_pool(name="o", bufs=N_OC) as opool,
            tc.tile_pool(name="ps", bufs=N_TILES, space="PSUM") as pspool,
        ):
            consts_sb = const_pool.tile([DEP_NUM, F + RPC], bf16)
            ce = cfg["consts_eng"]
            if "," in ce or cfg.get("consts_split"):
                # two parallel DMAs: table and one-hot halves
                eng(ce, 0).dma_start(
                    out=consts_sb[:, :F], in_=consts_in[:, :F]
                )
                eng(ce, 1).dma_start(
                    out=consts_sb[:, F:], in_=consts_in[:, F:]
                )
            else:
                eng(ce).dma_start(out=consts_sb[:], in_=consts_in[:])
            tab_r = consts_sb[:, :F]
            oh_sb = consts_sb[:, F:]

            xs = []
            for c in range(N_IC):
                x = xpool.tile([P, IC * F], f32)
                eng(cfg.get("in_eng", "sync"), c).dma_start(out=x[:], in_=text_v[c])
                for u in range(IC):
                    xs.append(x[:, bass.ts(u, F)])

            for c in range(N_OC):
                o = opool.tile([P, OC * F], f32)
                for u in range(OC):
                    t = c * OC + u
                    ps = pspool.tile([P, F], f32)
                    nc.tensor.matmul(
                        out=ps[:],
                        lhsT=oh_sb[:, bass.ts(t, P)],
                        rhs=tab_r,
                        start=True,
                        stop=True,
                    )
                    osl = o[:, bass.ts(u, F)]
                    nc.vector.tensor_add(osl, xs[t], ps[:])
                    relu_eng = cfg["relu_eng"]
                    if relu_eng == "split":
                        relu_eng = "scalar" if u % 2 == 0 else "vector"
                    if relu_eng == "vector":
                        nc.vector.tensor_scalar_max(osl, osl, 0.0)
                    else:
                        nc.scalar.activation(
                            osl, osl, mybir.ActivationFunctionType.Relu
                        )

                eng(cfg["out_eng"], c).dma_start(out=out_v[c], in_=o[:])

    nc.compile()
    return nc


def prepare_in_maps(text, dep_labels, emb, fc_w, fc_b):
    """Host-side prep: table = emb @ fc_w + fc_b, one-hot labels, row shards."""
    text = np.asarray(text, dtype=np.float32)
    labels = np.asarray(dep_labels, dtype=np.int32)
    emb = np.asarray(emb, dtype=np.float32)
    fc_w = np.asarray(fc_w, dtype=np.float32)
    fc_b = np.asarray(fc_b, dtype=np.float32)

    import ml_dtypes

    table = (emb @ fc_w + fc_b).astype(np.float32)           # [50, F]
    flat_text = np.ascontiguousarray(text.reshape(ROWS, F))
    flat_labels = labels.reshape(ROWS)
    onehot = flat_labels[:, None] == np.arange(DEP_NUM, dtype=np.int32)[None, :]

    in_maps = []
    for c in range(N_CORES):
        rows = slice(c * RPC, (c + 1) * RPC)
        oh_t = onehot[rows].T.astype(np.float32)             # [50, RPC]
        consts = np.concatenate([table, oh_t], axis=1)       # [50, F + RPC]
        in_maps.append(
            {
                "text": flat_text[rows],
                "consts": np.ascontiguousarray(consts).astype(ml_dtypes.bfloat16),
            }
        )
    return in_maps


def assemble_output(per_core_outs):
    out = np.concatenate(list(per_core_outs), axis=0)
    return out.reshape(B, S, F).astype(np.float32)


def kernel(text, dep_mat, dep_labels, emb, attn_w, attn_b, fc_w, fc_b):
    global last_results

    in_maps = prepare_in_maps(text, dep_labels, emb, fc_w, fc_b)
    nc = _build_program()
    res = run_bass_kernel_spmd(nc, in_maps, list(range(N_CORES)))
    last_results = res

    return assemble_output(res.results[c]["out"] for c in range(N_CORES))
